# revision 2
# baseline (speedup 1.0000x reference)
"""Trainium2 Bass kernel for windowed multi-agent attention (Swin-style).

Full-input contract: kernel(**inputs) takes the unsharded inputs and returns
the unsharded output. Internally shards over the H axis across 8 NeuronCores
(fully data-parallel over window rows), builds one SPMD Bass program, and
runs it via run_bass_kernel_spmd.

Math (per window of 4x4 spatial, 4 agents => T=64 tokens; the 5th padded
agent is masked out everywhere in the reference, so it is simply dropped):
  xw (64, 256) -> qkv -> 4 heads of d=64 -> softmax(q k^T * scale + bias) v
  -> out proj (256, 256) -> back to NCHW.

v2 layout notes:
  - All I/O in bf16 (host casts); per strip 2 input + 2 output DMAs with
    1KB contiguous runs (full DMA rate, minimal HWDGE occupancy).
  - One packed const DMA (wq|wk|wv|wo|bias|ident) in bf16.
  - PSUM staged through two-bank (128,1024) f32 tiles: Q|K, V, S(A|B),
    O(A|B), U(A|B); transposes through one-bank (128,1024) bf16 tiles.
  - Elementwise/evac ops spread across Act/DVE/Pool (env-tunable).
"""

import os
import numpy as np

HEADS = 4
WIN = 4
MAX_N = 5
DIM = 256
N_AGENTS = 4
H = W = 128
N_CORES = 8
T = N_AGENTS * WIN * WIN          # 64 valid tokens per window
HS = 16                           # H rows per core
N_STRIPS = 4                      # window-rows per core (4 H-rows each)
N_GROUPS = 4                      # groups of 8 windows per strip
GW = 8                            # windows per group
NT = GW * T                       # tokens per group = 512


def _rel_pos_index(N, wh, ww, md, mh, mw):
    cd, ch, cw = np.arange(N), np.arange(wh), np.arange(ww)
    coords = np.stack(np.meshgrid(cd, ch, cw, indexing="ij")).reshape(3, -1)
    rel = (coords[:, :, None] - coords[:, None, :]).transpose(1, 2, 0).astype(np.int64)
    rel[..., 0] += md - 1
    rel[..., 1] += mh - 1
    rel[..., 2] += mw - 1
    rel[..., 0] *= (2 * mh - 1) * (2 * mw - 1)
    rel[..., 1] *= 2 * mw - 1
    return rel.sum(-1)


def _build_bias_stack(bias_table):
    """exp(bias) as one (128, 1024) stack: cols = [stack A (512) | stack B
    (512)], each stack = heads (2s, 2s+1) on partition halves x 8 windows
    tiled x t_k."""
    rpi = _rel_pos_index(MAX_N, WIN, WIN, MAX_N, WIN, WIN)  # (80, 80)
    b = bias_table[rpi]                                     # (80, 80, HEADS)
    b = b[:T, :T].transpose(2, 0, 1).astype(np.float32)     # (HEADS, 64, 64)
    stacks = []
    for s in range(2):
        st = np.concatenate([b[2 * s], b[2 * s + 1]], axis=0)   # (128, 64)
        stacks.append(np.tile(np.exp(st), (1, GW)))             # (128, 512)
    return np.concatenate(stacks, axis=1)                       # (128, 1024)


def _patch_tile_drain():
    """Walrus in this container rejects >1 sync-wait on the TileContext tail
    drain; split the waits across individual SP nops instead."""
    from concourse import tile as tile_mod
    from concourse.vector_clock import ScopedClock, VectorClock
    if getattr(tile_mod.TileContext, "_drain_patched", False):
        return

    def _patched(self, tick_clock, wait_clock):
        gc_ = tick_clock.global_clock
        n = len(gc_)
        for proc in range(n):
            tick = gc_[proc]
            if tick <= 0:
                continue
            vc = VectorClock([0] * n)
            vc.require_at_least(proc, tick)
            nop_inst = self.nc.sync.nop(nofuse=True)
            wait_clock.add_sem_waits(nop_inst.ins, ScopedClock({None: vc}))
        self.nc.sync.drain()
        self.nc.all_engine_barrier()
        popped = self.nc._tile_sem_poison_stack.pop()
        assert popped is self._sem_poison
        self.nc.clear_and_free_semaphores(list(self.sems.allocated().values()))
        self.nc.all_engine_barrier()

    tile_mod.TileContext._drain_and_barrier = _patched
    tile_mod.TileContext._drain_patched = True


def _split_multi_waits(nc):
    """Walrus here allows only one sync-wait per instruction. Rewrite the BIR
    json: for each instruction with >1 on_wait, hoist the extras onto fresh
    single-wait Nops inserted just before it on the same engine."""
    import orjson
    orig = nc.to_json_bytes

    def patched():
        bj = orjson.loads(orig())
        counter = [0]
        for fn in bj.get("functions", []):
            for blk in fn.get("blocks", []):
                insts = blk.get("instructions", [])
                out = []
                for inst in insts:
                    si = inst.get("sync_info") or {}
                    waits = si.get("on_wait") or []
                    if len(waits) > 1:
                        for w in waits[:-1]:
                            counter[0] += 1
                            out.append({
                                "name": f"WSPL-{counter[0]}",
                                "opcode": "NoOp",
                                "engine": inst["engine"],
                                "ins": [],
                                "outs": [],
                                "sync_info": {"on_update": [], "on_wait": [w]},
                            })
                        si["on_wait"] = [waits[-1]]
                    out.append(inst)
                blk["instructions"] = out
        return orjson.dumps(bj)

    nc.to_json_bytes = patched
    return nc


# const layout offsets (cols in the packed (128, NCONST) const tile)
_OFF_WQ = 0          # 4 x 128: (c0h0, c0h1, c1h0, c1h1)
_OFF_WK = 512
_OFF_WV = 1024       # 2 x 256: (c0, c1)
_OFF_WO = 1536       # 4 x 128
_OFF_BIAS = 2048     # 1024
_OFF_IDENT = 3072    # 128
NCONST = 3200


def build_nc():
    from concourse import bass, mybir
    from concourse.tile import TileContext
    _patch_tile_drain()

    # engine assignment knobs: each is one of "act", "dve", "pool"
    ASG = {
        "qev": os.environ.get("KQEV", "act"),    # Q psum->sbuf
        "kev": os.environ.get("KKEV", "act"),    # K psum->sbuf
        "vev": os.environ.get("KVEV", "pool"),   # V psum->sbuf
        "norm": os.environ.get("KNORM", "dve"),  # softmax normalize mul
        "atev": os.environ.get("KATEV", "dve"),  # attn-transpose evac
        "orev": os.environ.get("KOREV", "pool"), # av-out evac
        "otev": os.environ.get("KOTEV", "dve"),  # o-transpose evac
        "osev": os.environ.get("KOSEV", "act"),  # out-proj evac
        "irdr": os.environ.get("KIRDR", "dve"),  # input reorder
        "ordr": os.environ.get("KORDR", "dve"),  # output reorder
    }
    STAGE = os.environ.get("KSTAGE", "full")

    def stage_ge(s):
        order = ["qkv", "sim", "soft", "tp", "av", "out", "full"]
        return order.index(STAGE) >= order.index(s)

    F32 = mybir.dt.float32
    BF16 = mybir.dt.bfloat16
    AX = mybir.AxisListType.X
    EXP = mybir.ActivationFunctionType.Exp

    nc = bass.Bass("TRN2", target_bir_lowering=False, debug=False,
                   num_devices=N_CORES)

    def eng(which):
        return {"act": nc.scalar, "dve": nc.vector, "pool": nc.gpsimd}[ASG[which]]

    def ecopy(which, dst, src):
        e = eng(which)
        if e is nc.scalar:
            e.copy(dst, src)
        else:
            e.tensor_copy(dst, src)

    xs_d = nc.dram_tensor("xs", [N_AGENTS, DIM, HS, W], BF16, kind="ExternalInput").ap()
    cn_d = nc.dram_tensor("consts", [128, NCONST], BF16, kind="ExternalInput").ap()
    out_d = nc.dram_tensor("out", [N_AGENTS, DIM, HS, W], BF16, kind="ExternalOutput").ap()

    from contextlib import ExitStack
    with TileContext(nc) as tc, ExitStack() as _stk:
        strip = _stk.enter_context(tc.tile_pool(name="strip", bufs=2))
        # issue strip-0 input DMAs before the const DMA so tokens stream first
        Traw0 = [strip.tile([128, 2048], BF16, name=f"traw{c}", tag=f"traw{c}") for c in range(2)]
        for c in range(2):
            src = xs_d[:, c * 128:(c + 1) * 128, 0:WIN, :].transpose([1, 0, 2, 3])
            nc.sync.dma_start(out=Traw0[c][:].rearrange("p (a i w) -> p a i w", a=4, i=4), in_=src)

        cpool = _stk.enter_context(tc.tile_pool(name="consts", bufs=1))
        CN = cpool.tile([128, NCONST], BF16, name="cn", tag="cn")
        nc.sync.dma_start(out=CN[:], in_=cn_d)
        wq = [[CN[:, _OFF_WQ + (2 * c + h) * 128: _OFF_WQ + (2 * c + h + 1) * 128] for h in range(2)] for c in range(2)]
        wk = [[CN[:, _OFF_WK + (2 * c + h) * 128: _OFF_WK + (2 * c + h + 1) * 128] for h in range(2)] for c in range(2)]
        wv = [CN[:, _OFF_WV + c * 256: _OFF_WV + (c + 1) * 256] for c in range(2)]
        wo = [[CN[:, _OFF_WO + (2 * c + h) * 128: _OFF_WO + (2 * c + h + 1) * 128] for h in range(2)] for c in range(2)]
        biasAB = CN[:, _OFF_BIAS: _OFF_BIAS + 1024]
        ident = CN[:, _OFF_IDENT: _OFF_IDENT + 128]

        grp = _stk.enter_context(tc.tile_pool(name="grp", bufs=3))
        psB = _stk.enter_context(tc.tile_pool(name="psB", bufs=3, space="PSUM"))
        psT = _stk.enter_context(tc.tile_pool(name="psT", bufs=2, space="PSUM"))

        def load_strip(s, first=False):
            hs_sl = slice(s * WIN, (s + 1) * WIN)
            if first:
                Traw = Traw0
            else:
                Traw = [strip.tile([128, 2048], BF16, name=f"traw{c}", tag=f"traw{c}") for c in range(2)]
                for c in range(2):
                    src = xs_d[:, c * 128:(c + 1) * 128, hs_sl, :].transpose([1, 0, 2, 3])
                    nc.sync.dma_start(
                        out=Traw[c][:].rearrange("p (a i w) -> p a i w", a=4, i=4), in_=src)
            Ttok = [strip.tile([128, 2048], BF16, name=f"ttok{c}", tag=f"ttok{c}") for c in range(2)]
            for c in range(2):
                tokv = Ttok[c][:].rearrange("p (w a i j) -> p a w i j", w=32, a=4, i=4, j=4)
                rawv = Traw[c][:].rearrange("p (a i w j) -> p a w i j", a=4, i=4, w=32, j=4)
                for a_ in range(4):
                    eng("irdr").tensor_copy(tokv[:, a_], rawv[:, a_])
            return Ttok

        pending = load_strip(0, first=True)
        for s in range(N_STRIPS):
            hs_sl = slice(s * WIN, (s + 1) * WIN)
            Ttok = pending
            if s + 1 < N_STRIPS:
                pending = load_strip(s + 1)
            OS = [strip.tile([128, 2048], BF16, name=f"os{c}", tag=f"os{c}") for c in range(2)]
            OR = [strip.tile([128, 2048], BF16, name=f"or{c}", tag=f"or{c}") for c in range(2)]

            for g in range(N_GROUPS):
                gt = slice(g * NT, (g + 1) * NT)
                tok = [Ttok[c][:, gt] for c in range(2)]

                # ---- q, k projections: Qp = (QA | QB), heads pair-stacked ----
                Qp = psB.tile([128, 1024], F32, name="Qp", tag="psB")
                Kp = psB.tile([128, 1024], F32, name="Kp", tag="psB")
                for h in range(2):
                    hsl = slice(h * 512, (h + 1) * 512)
                    for c in range(2):
                        nc.tensor.matmul(Qp[:, hsl], wq[c][h], tok[c], start=(c == 0), stop=(c == 1))
                        nc.tensor.matmul(Kp[:, hsl], wk[c][h], tok[c], start=(c == 0), stop=(c == 1))
                # ---- v (token-rows), Vp = 4 window-pair blocks of (128,256) ----
                Vp = psB.tile([128, 1024], F32, name="Vp", tag="psB")
                for p in range(4):
                    for c in range(2):
                        lhsT = Ttok[c][:, g * NT + p * 128: g * NT + (p + 1) * 128]
                        nc.tensor.matmul(Vp[:, p * 256:(p + 1) * 256], lhsT, wv[c], start=(c == 0), stop=(c == 1))
                qA = grp.tile([128, 1024], BF16, name="qA", tag="qA")
                kA = grp.tile([128, 1024], BF16, name="kA", tag="kA")
                vS = grp.tile([128, 1024], BF16, name="vS", tag="vS")
                ecopy("qev", qA[:], Qp[:])
                ecopy("kev", kA[:], Kp[:])
                ecopy("vev", vS[:], Vp[:])

                if not stage_ge("sim"):
                    ecopy("osev", OS[0][:, gt], qA[:, 0:512])
                    ecopy("osev", OS[1][:, gt], kA[:, 0:512])
                    continue
                # ---- sim: Sp = (SA | SB): partitions (hh, t_q), cols (w, t_k) ----
                Sp = psB.tile([128, 1024], F32, name="Sp", tag="psB")
                for st in range(2):
                    for w in range(GW):
                        for hh in range(2):
                            pp = slice(hh * 64, (hh + 1) * 64)
                            cs = slice(st * 512 + w * 64, st * 512 + (w + 1) * 64)
                            nc.tensor.matmul(Sp[pp, cs], qA[pp, cs], kA[pp, cs], start=True, stop=True)

                if not stage_ge("soft"):
                    ecopy("osev", OS[0][:, gt], Sp[:, 0:512])
                    ecopy("osev", OS[1][:, gt], Sp[:, 512:1024])
                    continue
                # ---- softmax over t_k (free axis) ----
                Eu = grp.tile([128, 1024], BF16, name="Eu", tag="Eu")
                EB = grp.tile([128, 1024], BF16, name="EB", tag="EB")
                rs = grp.tile([128, 16], F32, name="rs", tag="rs")
                rr = grp.tile([128, 16], F32, name="rr", tag="rr")
                NN = grp.tile([128, 1024], BF16, name="NN", tag="NN")
                nc.scalar.activation(Eu[:], Sp[:], EXP)
                nc.vector.tensor_mul(EB[:], Eu[:], biasAB)
                nc.vector.reduce_sum(rs[:], EB[:].rearrange("p (u k) -> p u k", u=16), axis=AX)
                nc.vector.reciprocal(rr[:], rs[:])
                eng("norm").tensor_mul(
                    NN[:].rearrange("p (u k) -> p u k", u=16),
                    EB[:].rearrange("p (u k) -> p u k", u=16),
                    rr[:].unsqueeze(2).broadcast_to([128, 16, T]),
                )

                if not stage_ge("tp"):
                    ecopy("osev", OS[0][:, gt], NN[:, 0:512])
                    ecopy("osev", OS[1][:, gt], NN[:, 512:1024])
                    continue
                # ---- transpose attn -> (wl*64+t_k, st|p|hh|t_q) ----
                Tp = psT.tile([128, 1024], BF16, name="Tp", tag="psT")
                for b in range(8):
                    isl = slice(b * 128, (b + 1) * 128)
                    nc.tensor.transpose(Tp[:, isl], NN[:, isl], ident)
                aT = grp.tile([128, 1024], BF16, name="aT", tag="aT")
                ecopy("atev", aT[:], Tp[:])

                if not stage_ge("av"):
                    ecopy("osev", OS[0][:, gt], aT[:, 0:512])
                    ecopy("osev", OS[1][:, gt], aT[:, 512:1024])
                    continue
                # ---- attn @ v: Op partitions (wl, t_q), cols (st, p, hh, d) ----
                Op = psB.tile([128, 1024], F32, name="Op", tag="psB")
                for st in range(2):
                    for p in range(4):
                        for wl in range(2):
                            ksl = slice(wl * 64, (wl + 1) * 64)
                            for hh in range(2):
                                csl = slice(st * 512 + p * 128 + hh * 64,
                                            st * 512 + p * 128 + (hh + 1) * 64)
                                nc.tensor.matmul(
                                    Op[ksl, csl],
                                    aT[ksl, st * 512 + p * 128 + hh * 64: st * 512 + p * 128 + (hh + 1) * 64],
                                    vS[ksl, p * 256 + st * 128 + hh * 64: p * 256 + st * 128 + (hh + 1) * 64],
                                    start=True, stop=True)
                oR = grp.tile([128, 1024], BF16, name="oR", tag="oR")
                ecopy("orev", oR[:], Op[:])

                # ---- transpose o -> oT: partitions (hh, d) per c-half, cols (p, wl, t_q) ----
                TPp = psT.tile([128, 1024], BF16, name="TPp", tag="psT")
                for b in range(8):
                    isl = slice(b * 128, (b + 1) * 128)
                    nc.tensor.transpose(TPp[:, isl], oR[:, isl], ident)
                oT = grp.tile([128, 1024], BF16, name="oT", tag="oT")
                ecopy("otev", oT[:], TPp[:])

                if not stage_ge("out"):
                    ecopy("osev", OS[0][:, gt], oT[:, 0:512])
                    ecopy("osev", OS[1][:, gt], oT[:, 512:1024])
                    continue
                # ---- out projection: Up = (UA | UB) = (cout, tokens) ----
                Up = psB.tile([128, 1024], F32, name="Up", tag="psB")
                for st in range(2):
                    o_sl = oT[:, st * 512:(st + 1) * 512]
                    nc.tensor.matmul(Up[:, 0:512], wo[st][0], o_sl, start=(st == 0), stop=(st == 1))
                    nc.tensor.matmul(Up[:, 512:1024], wo[st][1], o_sl, start=(st == 0), stop=(st == 1))
                ecopy("osev", OS[0][:, gt], Up[:, 0:512])
                ecopy("osev", OS[1][:, gt], Up[:, 512:1024])

            # ---- reorder (w a i j) -> (a i w j), DMA out ----
            for c in range(2):
                orv = OR[c][:].rearrange("p (a i w j) -> p a w i j", a=4, i=4, w=32, j=4)
                osv = OS[c][:].rearrange("p (w a i j) -> p a w i j", w=32, a=4, i=4, j=4)
                for a_ in range(4):
                    eng("ordr").tensor_copy(orv[:, a_], osv[:, a_])
                dst = out_d[:, c * 128:(c + 1) * 128, hs_sl, :].transpose([1, 0, 2, 3])
                nc.sync.dma_start(
                    out=dst, in_=OR[c][:].rearrange("p (a i w) -> p a i w", a=4, i=4))

    return _split_multi_waits(nc)


_NC_CACHE = None


def kernel(x, w_qkv, w_out, bias_table, _want_trace=False):
    global _NC_CACHE
    from concourse.bass_utils import run_bass_kernel_spmd
    import ml_dtypes

    x = np.asarray(x, dtype=np.float32)
    w_qkv = np.asarray(w_qkv, dtype=np.float32)
    w_out = np.asarray(w_out, dtype=np.float32)
    bias_table = np.asarray(bias_table, dtype=np.float32)

    scale = (DIM // HEADS) ** -0.5
    wq = w_qkv[:, 0:DIM] * scale
    wk = w_qkv[:, DIM:2 * DIM]
    wv = w_qkv[:, 2 * DIM:3 * DIM]

    # packed const tile (128, NCONST)
    cn = np.zeros((128, NCONST), dtype=np.float32)
    for c in range(2):
        cs = slice(c * 128, (c + 1) * 128)
        for h in range(2):
            hs_ = slice(h * 128, (h + 1) * 128)
            cn[:, _OFF_WQ + (2 * c + h) * 128: _OFF_WQ + (2 * c + h + 1) * 128] = wq[cs, hs_]
            cn[:, _OFF_WK + (2 * c + h) * 128: _OFF_WK + (2 * c + h + 1) * 128] = wk[cs, hs_]
            cn[:, _OFF_WO + (2 * c + h) * 128: _OFF_WO + (2 * c + h + 1) * 128] = w_out[cs, hs_]
        cn[:, _OFF_WV + c * 256: _OFF_WV + (c + 1) * 256] = wv[cs, :]
    cn[:, _OFF_BIAS: _OFF_BIAS + 1024] = _build_bias_stack(bias_table)
    cn[:, _OFF_IDENT: _OFF_IDENT + 128] = np.eye(128, dtype=np.float32)
    cn = cn.astype(ml_dtypes.bfloat16)

    if _NC_CACHE is None:
        _NC_CACHE = build_nc()
    nc = _NC_CACHE

    xb = x.astype(ml_dtypes.bfloat16)
    in_maps = []
    for m in range(N_CORES):
        xs = np.ascontiguousarray(xb[:, :, m * HS:(m + 1) * HS, :])
        in_maps.append({"xs": xs, "consts": cn})
    res = run_bass_kernel_spmd(nc, in_maps, list(range(N_CORES)), trace=_want_trace)
    out = np.empty((N_AGENTS, DIM, H, W), dtype=np.float32)
    for m in range(N_CORES):
        out[:, :, m * HS:(m + 1) * HS, :] = np.asarray(res.results[m]["out"], dtype=np.float32)
    if _want_trace:
        return out, res
    return out


# revision 3
# speedup vs baseline: 2.0616x; 2.0616x over previous
"""Trainium2 Bass kernel for windowed multi-agent attention (Swin-style).

Full-input contract: kernel(**inputs) takes the unsharded inputs and returns
the unsharded output. Internally shards over the H axis across 8 NeuronCores
(fully data-parallel over window rows), builds one SPMD Bass program, and
runs it via run_bass_kernel_spmd.

Math (per window of 4x4 spatial, 4 agents => T=64 tokens; the 5th padded
agent is masked out everywhere in the reference, so it is simply dropped):
  xw (64, 256) -> qkv -> 4 heads of d=64 -> softmax(q k^T * scale + bias) v
  -> out proj (256, 256) -> back to NCHW.

v3 structure:
  - bf16 I/O (host casts), 2 big DMAs per strip per direction with 1KB
    contiguous runs; one packed const DMA.
  - Software-pipelined flat group loop: iteration g emits
    qkv(g) | transpose+av(g-1) | sim+softmax(g) | outproj(g-2)
    so softmax/evac latency of one group hides under PE work of others.
  - av computes o in transposed (c-major) form directly -> no second
    transpose stage.
  - All PSUM tiles are 2KB/partition (one bank) rotating through one
    8-slot pool.
"""

import os
import numpy as np

HEADS = 4
WIN = 4
MAX_N = 5
DIM = 256
N_AGENTS = 4
H = W = 128
N_CORES = 8
T = N_AGENTS * WIN * WIN          # 64 valid tokens per window
HS = 16                           # H rows per core
N_STRIPS = 4                      # window-rows per core (4 H-rows each)
N_GROUPS = 4                      # groups of 8 windows per strip
NG = N_STRIPS * N_GROUPS          # 16 groups, flattened
GW = 8                            # windows per group
NT = GW * T                       # tokens per group = 512


def _rel_pos_index(N, wh, ww, md, mh, mw):
    cd, ch, cw = np.arange(N), np.arange(wh), np.arange(ww)
    coords = np.stack(np.meshgrid(cd, ch, cw, indexing="ij")).reshape(3, -1)
    rel = (coords[:, :, None] - coords[:, None, :]).transpose(1, 2, 0).astype(np.int64)
    rel[..., 0] += md - 1
    rel[..., 1] += mh - 1
    rel[..., 2] += mw - 1
    rel[..., 0] *= (2 * mh - 1) * (2 * mw - 1)
    rel[..., 1] *= 2 * mw - 1
    return rel.sum(-1)


def _build_bias_stack(bias_table):
    """exp(bias) as one (128, 1024) stack: cols = [stack A (512) | stack B
    (512)], each stack = heads (2s, 2s+1) on partition halves x 8 windows
    tiled x t_k."""
    rpi = _rel_pos_index(MAX_N, WIN, WIN, MAX_N, WIN, WIN)  # (80, 80)
    b = bias_table[rpi]                                     # (80, 80, HEADS)
    b = b[:T, :T].transpose(2, 0, 1).astype(np.float32)     # (HEADS, 64, 64)
    stacks = []
    for s in range(2):
        st = np.concatenate([b[2 * s], b[2 * s + 1]], axis=0)   # (128, 64)
        stacks.append(np.tile(np.exp(st), (1, GW)))             # (128, 512)
    return np.concatenate(stacks, axis=1)                       # (128, 1024)


def _patch_tile_drain():
    """Walrus in this container rejects >1 sync-wait on the TileContext tail
    drain; split the waits across individual SP nops instead."""
    from concourse import tile as tile_mod
    from concourse.vector_clock import ScopedClock, VectorClock
    if getattr(tile_mod.TileContext, "_drain_patched", False):
        return

    def _patched(self, tick_clock, wait_clock):
        gc_ = tick_clock.global_clock
        n = len(gc_)
        for proc in range(n):
            tick = gc_[proc]
            if tick <= 0:
                continue
            vc = VectorClock([0] * n)
            vc.require_at_least(proc, tick)
            nop_inst = self.nc.sync.nop(nofuse=True)
            wait_clock.add_sem_waits(nop_inst.ins, ScopedClock({None: vc}))
        self.nc.sync.drain()
        self.nc.all_engine_barrier()
        popped = self.nc._tile_sem_poison_stack.pop()
        assert popped is self._sem_poison
        self.nc.clear_and_free_semaphores(list(self.sems.allocated().values()))
        self.nc.all_engine_barrier()

    tile_mod.TileContext._drain_and_barrier = _patched
    tile_mod.TileContext._drain_patched = True


def _split_multi_waits(nc):
    """Walrus here allows only one sync-wait per instruction. Rewrite the BIR
    json: for each instruction with >1 on_wait, hoist the extras onto fresh
    single-wait Nops inserted just before it on the same engine."""
    import orjson
    orig = nc.to_json_bytes

    def patched():
        bj = orjson.loads(orig())
        counter = [0]
        for fn in bj.get("functions", []):
            for blk in fn.get("blocks", []):
                insts = blk.get("instructions", [])
                out = []
                for inst in insts:
                    si = inst.get("sync_info") or {}
                    waits = si.get("on_wait") or []
                    if len(waits) > 1:
                        for w in waits[:-1]:
                            counter[0] += 1
                            out.append({
                                "name": f"WSPL-{counter[0]}",
                                "opcode": "NoOp",
                                "engine": inst["engine"],
                                "ins": [],
                                "outs": [],
                                "sync_info": {"on_update": [], "on_wait": [w]},
                            })
                        si["on_wait"] = [waits[-1]]
                    out.append(inst)
                blk["instructions"] = out
        return orjson.dumps(bj)

    nc.to_json_bytes = patched
    return nc


# const layout offsets (cols in the packed (128, NCONST) const tile)
_OFF_WQ = 0          # 4 x 128: (c0h0, c0h1, c1h0, c1h1)
_OFF_WK = 512
_OFF_WV = 1024       # 2 x 256: (c0, c1)
_OFF_WO = 1536       # 4 x 128
_OFF_BIAS = 2048     # 1024
_OFF_IDENT = 3072    # 128
NCONST = 3200


def build_nc():
    from concourse import bass, mybir
    from concourse.tile import TileContext
    _patch_tile_drain()

    # engine assignment knobs: each is one of "act", "dve", "pool"
    ASG = {
        "qev": os.environ.get("KQEV", "act"),    # Q psum->sbuf (x2)
        "kev": os.environ.get("KKEV", "act"),    # K psum->sbuf (x2)
        "vev": os.environ.get("KVEV", "dve"),    # V psum->sbuf (x2)
        "norm": os.environ.get("KNORM", "pool"), # softmax normalize mul (x2)
        "atev": os.environ.get("KATEV", "dve"),  # attn-transpose evac (x1)
        "otev": os.environ.get("KOTEV", "pool"), # oT psum->sbuf (x2)
        "osev": os.environ.get("KOSEV", "act"),  # out-proj evac (x2)
        "irdr": os.environ.get("KIRDR", "dve"),  # input reorder
        "ordr": os.environ.get("KORDR", "dve"),  # output reorder
    }

    F32 = mybir.dt.float32
    BF16 = mybir.dt.bfloat16
    AX = mybir.AxisListType.X
    EXP = mybir.ActivationFunctionType.Exp

    nc = bass.Bass("TRN2", target_bir_lowering=False, debug=False,
                   num_devices=N_CORES)

    def eng(which):
        return {"act": nc.scalar, "dve": nc.vector, "pool": nc.gpsimd}[ASG[which]]

    def ecopy(which, dst, src):
        e = eng(which)
        if e is nc.scalar:
            e.copy(dst, src)
        else:
            e.tensor_copy(dst, src)

    xs_d = nc.dram_tensor("xs", [N_AGENTS, DIM, HS, W], BF16, kind="ExternalInput").ap()
    cn_d = nc.dram_tensor("consts", [128, NCONST], BF16, kind="ExternalInput").ap()
    out_d = nc.dram_tensor("out", [N_AGENTS, DIM, HS, W], BF16, kind="ExternalOutput").ap()

    from contextlib import ExitStack
    with TileContext(nc) as tc, ExitStack() as _stk:
        strip = _stk.enter_context(tc.tile_pool(name="strip", bufs=2))
        # issue strip-0 input DMAs before the const DMA so tokens stream first
        Traw0 = [strip.tile([128, 2048], BF16, name=f"traw{c}", tag=f"traw{c}") for c in range(2)]
        for c in range(2):
            src = xs_d[:, c * 128:(c + 1) * 128, 0:WIN, :].transpose([1, 0, 2, 3])
            nc.sync.dma_start(out=Traw0[c][:].rearrange("p (a i w) -> p a i w", a=4, i=4), in_=src)

        cpool = _stk.enter_context(tc.tile_pool(name="consts", bufs=1))
        CN = cpool.tile([128, NCONST], BF16, name="cn", tag="cn")
        nc.sync.dma_start(out=CN[:], in_=cn_d)
        wq = [[CN[:, _OFF_WQ + (2 * c + h) * 128: _OFF_WQ + (2 * c + h + 1) * 128] for h in range(2)] for c in range(2)]
        wk = [[CN[:, _OFF_WK + (2 * c + h) * 128: _OFF_WK + (2 * c + h + 1) * 128] for h in range(2)] for c in range(2)]
        wv = [CN[:, _OFF_WV + c * 256: _OFF_WV + (c + 1) * 256] for c in range(2)]
        wo = [[CN[:, _OFF_WO + (2 * c + h) * 128: _OFF_WO + (2 * c + h + 1) * 128] for h in range(2)] for c in range(2)]
        biasAB = CN[:, _OFF_BIAS: _OFF_BIAS + 1024]
        ident = CN[:, _OFF_IDENT: _OFF_IDENT + 128]

        grp = _stk.enter_context(tc.tile_pool(name="grp", bufs=3))
        ps = _stk.enter_context(tc.tile_pool(name="ps", bufs=8, space="PSUM"))

        # per-flattened-group state
        st_tok = {}    # strip -> Ttok pair
        st_os = {}     # strip -> OS pair
        gctx = {}      # g -> dict of tiles

        def emit_strip_dma(s):
            Traw = Traw0 if s == 0 else [
                strip.tile([128, 2048], BF16, name=f"traw{c}", tag=f"traw{c}") for c in range(2)]
            if s > 0:
                hs_sl = slice(s * WIN, (s + 1) * WIN)
                for c in range(2):
                    src = xs_d[:, c * 128:(c + 1) * 128, hs_sl, :].transpose([1, 0, 2, 3])
                    nc.sync.dma_start(
                        out=Traw[c][:].rearrange("p (a i w) -> p a i w", a=4, i=4), in_=src)
            return Traw

        def emit_strip_reorder(s, Traw, half):
            """half 0/1: reorder two agents each for both c (4 ops)."""
            if s not in st_tok:
                st_tok[s] = [strip.tile([128, 2048], BF16, name=f"ttok{c}", tag=f"ttok{c}") for c in range(2)]
            Ttok = st_tok[s]
            for c in range(2):
                tokv = Ttok[c][:].rearrange("p (w a i j) -> p a w i j", w=32, a=4, i=4, j=4)
                rawv = Traw[c][:].rearrange("p (a i w j) -> p a w i j", a=4, i=4, w=32, j=4)
                for a_ in (2 * half, 2 * half + 1):
                    eng("irdr").tensor_copy(tokv[:, a_], rawv[:, a_])

        def phase_qkv(g):
            s, gi = divmod(g, N_GROUPS)
            Ttok = st_tok[s]
            gt = slice(gi * NT, (gi + 1) * NT)
            tok = [Ttok[c][:, gt] for c in range(2)]
            Qp = [ps.tile([128, 512], F32, name=f"Qp{h}", tag="ps") for h in range(2)]
            Kp = [ps.tile([128, 512], F32, name=f"Kp{h}", tag="ps") for h in range(2)]
            for h in range(2):
                for c in range(2):
                    nc.tensor.matmul(Qp[h][:], wq[c][h], tok[c], start=(c == 0), stop=(c == 1))
                    nc.tensor.matmul(Kp[h][:], wk[c][h], tok[c], start=(c == 0), stop=(c == 1))
            Vp = [ps.tile([128, 512], F32, name=f"Vp{i}", tag="ps") for i in range(2)]
            for p in range(4):
                for c in range(2):
                    lhsT = Ttok[c][:, gi * NT + p * 128: gi * NT + (p + 1) * 128]
                    nc.tensor.matmul(Vp[p // 2][:, (p % 2) * 256:(p % 2 + 1) * 256],
                                     lhsT, wv[c], start=(c == 0), stop=(c == 1))
            qA = grp.tile([128, 1024], BF16, name="qA", tag="qA")
            kA = grp.tile([128, 1024], BF16, name="kA", tag="kA")
            vS = grp.tile([128, 1024], BF16, name="vS", tag="vS")
            for h in range(2):
                hsl = slice(h * 512, (h + 1) * 512)
                ecopy("qev", qA[:, hsl], Qp[h][:])
                ecopy("kev", kA[:, hsl], Kp[h][:])
                ecopy("vev", vS[:, hsl], Vp[h][:])
            gctx[g] = {"qA": qA, "kA": kA, "vS": vS, "gt": gt, "s": s}

        def phase_sim(g):
            c_ = gctx[g]
            qA, kA = c_["qA"], c_["kA"]
            Sp = [ps.tile([128, 512], F32, name=f"Sp{st}", tag="ps") for st in range(2)]
            for st in range(2):
                for w in range(GW):
                    for hh in range(2):
                        pp = slice(hh * 64, (hh + 1) * 64)
                        cs = slice(st * 512 + w * 64, st * 512 + (w + 1) * 64)
                        ws = slice(w * 64, (w + 1) * 64)
                        nc.tensor.matmul(Sp[st][pp, ws], qA[pp, cs], kA[pp, cs], start=True, stop=True)
            NN = grp.tile([128, 1024], BF16, name="NN", tag="NN")
            # softmax per stack (shorter dependency chains, A/B overlap)
            for st in range(2):
                ssl = slice(st * 512, (st + 1) * 512)
                Eu = grp.tile([128, 512], BF16, name=f"Eu{st}", tag=f"Eu{st}")
                EB = grp.tile([128, 512], BF16, name=f"EB{st}", tag=f"EB{st}")
                rs = grp.tile([128, 8], F32, name=f"rs{st}", tag=f"rs{st}")
                rr = grp.tile([128, 8], F32, name=f"rr{st}", tag=f"rr{st}")
                nc.scalar.activation(Eu[:], Sp[st][:], EXP)
                nc.vector.tensor_mul(EB[:], Eu[:], biasAB[:, ssl])
                nc.vector.reduce_sum(rs[:], EB[:].rearrange("p (w k) -> p w k", w=8), axis=AX)
                nc.vector.reciprocal(rr[:], rs[:])
                eng("norm").tensor_mul(
                    NN[:, ssl].rearrange("p (w k) -> p w k", w=8),
                    EB[:].rearrange("p (w k) -> p w k", w=8),
                    rr[:].unsqueeze(2).broadcast_to([128, 8, T]),
                )
            c_["NN"] = NN

        def phase_tav(g):
            """transpose attn + av (direct transposed output)."""
            c_ = gctx[g]
            NN, vS = c_["NN"], c_["vS"]
            Tp = ps.tile([128, 1024], BF16, name="Tp", tag="ps")
            for b in range(8):
                isl = slice(b * 128, (b + 1) * 128)
                nc.tensor.transpose(Tp[:, isl], NN[:, isl], ident)
            aT = grp.tile([128, 1024], BF16, name="aT", tag="aT")
            ecopy("atev", aT[:], Tp[:])
            oTp = [ps.tile([128, 512], F32, name=f"oTp{st}", tag="ps") for st in range(2)]
            for st in range(2):
                for p in range(4):
                    for wl in range(2):
                        ksl = slice(wl * 64, (wl + 1) * 64)
                        for hh in range(2):
                            nc.tensor.matmul(
                                oTp[st][hh * 64:(hh + 1) * 64, p * 128 + wl * 64: p * 128 + (wl + 1) * 64],
                                vS[ksl, p * 256 + st * 128 + hh * 64: p * 256 + st * 128 + (hh + 1) * 64],
                                aT[ksl, st * 512 + p * 128 + hh * 64: st * 512 + p * 128 + (hh + 1) * 64],
                                start=True, stop=True)
            oT = grp.tile([128, 1024], BF16, name="oT", tag="oT")
            for st in range(2):
                ecopy("otev", oT[:, st * 512:(st + 1) * 512], oTp[st][:])
            c_["oT"] = oT

        def phase_proj(g):
            c_ = gctx[g]
            oT, gt, s = c_["oT"], c_["gt"], c_["s"]
            if s not in st_os:
                st_os[s] = [strip.tile([128, 2048], BF16, name=f"os{c}", tag=f"os{c}") for c in range(2)]
            OS = st_os[s]
            Up = [ps.tile([128, 512], F32, name=f"Up{h}", tag="ps") for h in range(2)]
            for st in range(2):
                o_sl = oT[:, st * 512:(st + 1) * 512]
                nc.tensor.matmul(Up[0][:], wo[st][0], o_sl, start=(st == 0), stop=(st == 1))
                nc.tensor.matmul(Up[1][:], wo[st][1], o_sl, start=(st == 0), stop=(st == 1))
            for h in range(2):
                ecopy("osev", OS[h][:, gt], Up[h][:])
            del gctx[g]

        def emit_out_strip(s):
            OS = st_os.pop(s)
            OR = [strip.tile([128, 2048], BF16, name=f"or{c}", tag=f"or{c}") for c in range(2)]
            hs_sl = slice(s * WIN, (s + 1) * WIN)
            for c in range(2):
                orv = OR[c][:].rearrange("p (a i w j) -> p a w i j", a=4, i=4, w=32, j=4)
                osv = OS[c][:].rearrange("p (w a i j) -> p a w i j", w=32, a=4, i=4, j=4)
                for a_ in range(4):
                    eng("ordr").tensor_copy(orv[:, a_], osv[:, a_])
                dst = out_d[:, c * 128:(c + 1) * 128, hs_sl, :].transpose([1, 0, 2, 3])
                nc.sync.dma_start(
                    out=dst, in_=OR[c][:].rearrange("p (a i w) -> p a i w", a=4, i=4))

        # prologue: strip 0 load + reorder
        traw_pend = {0: emit_strip_dma(0)}
        emit_strip_reorder(0, traw_pend[0], 0)
        emit_strip_reorder(0, traw_pend[0], 1)

        for g in range(NG + 2):
            s, gi = divmod(g, N_GROUPS)
            # input prefetch for next strip, spread across its first two groups
            if g < NG:
                if gi == 0 and s + 1 < N_STRIPS:
                    traw_pend[s + 1] = emit_strip_dma(s + 1)
                    emit_strip_reorder(s + 1, traw_pend[s + 1], 0)
                elif gi == 1 and s + 1 < N_STRIPS:
                    emit_strip_reorder(s + 1, traw_pend[s + 1], 1)
                    del traw_pend[s + 1]
                phase_qkv(g)
            if g - 1 >= 0 and g - 1 < NG:
                phase_tav(g - 1)
            if g < NG:
                phase_sim(g)
            if g - 2 >= 0:
                phase_proj(g - 2)
                if (g - 2) % N_GROUPS == N_GROUPS - 1:
                    emit_out_strip((g - 2) // N_GROUPS)

    return _split_multi_waits(nc)


_NC_CACHE = None


def kernel(x, w_qkv, w_out, bias_table, _want_trace=False):
    global _NC_CACHE
    from concourse.bass_utils import run_bass_kernel_spmd
    import ml_dtypes

    x = np.asarray(x, dtype=np.float32)
    w_qkv = np.asarray(w_qkv, dtype=np.float32)
    w_out = np.asarray(w_out, dtype=np.float32)
    bias_table = np.asarray(bias_table, dtype=np.float32)

    scale = (DIM // HEADS) ** -0.5
    wq = w_qkv[:, 0:DIM] * scale
    wk = w_qkv[:, DIM:2 * DIM]
    wv = w_qkv[:, 2 * DIM:3 * DIM]

    # packed const tile (128, NCONST)
    cn = np.zeros((128, NCONST), dtype=np.float32)
    for c in range(2):
        cs = slice(c * 128, (c + 1) * 128)
        for h in range(2):
            hs_ = slice(h * 128, (h + 1) * 128)
            cn[:, _OFF_WQ + (2 * c + h) * 128: _OFF_WQ + (2 * c + h + 1) * 128] = wq[cs, hs_]
            cn[:, _OFF_WK + (2 * c + h) * 128: _OFF_WK + (2 * c + h + 1) * 128] = wk[cs, hs_]
            cn[:, _OFF_WO + (2 * c + h) * 128: _OFF_WO + (2 * c + h + 1) * 128] = w_out[cs, hs_]
        cn[:, _OFF_WV + c * 256: _OFF_WV + (c + 1) * 256] = wv[cs, :]
    cn[:, _OFF_BIAS: _OFF_BIAS + 1024] = _build_bias_stack(bias_table)
    cn[:, _OFF_IDENT: _OFF_IDENT + 128] = np.eye(128, dtype=np.float32)
    cn = cn.astype(ml_dtypes.bfloat16)

    if _NC_CACHE is None:
        _NC_CACHE = build_nc()
    nc = _NC_CACHE

    xb = x.astype(ml_dtypes.bfloat16)
    in_maps = []
    for m in range(N_CORES):
        xs = np.ascontiguousarray(xb[:, :, m * HS:(m + 1) * HS, :])
        in_maps.append({"xs": xs, "consts": cn})
    res = run_bass_kernel_spmd(nc, in_maps, list(range(N_CORES)), trace=_want_trace)
    out = np.empty((N_AGENTS, DIM, H, W), dtype=np.float32)
    for m in range(N_CORES):
        out[:, :, m * HS:(m + 1) * HS, :] = np.asarray(res.results[m]["out"], dtype=np.float32)
    if _want_trace:
        return out, res
    return out
